# revision 9
# baseline (speedup 1.0000x reference)
"""Multi-head causal attention (B=2, T=2048, C=2048, H=16) on 8 TRN2 NeuronCores.

Sharding: data parallel over batch (2) x tensor parallel over head groups
(16 heads -> 4 groups of 4). Core c handles batch c//4, heads 4*(c%4)..4*(c%4)+3.
Each core computes its Megatron-style partial output projection; the host sums
the 4 partials per batch and adds the bias.

All matmuls are natural-layout because the host pre-transposes x and the
weights.  Scores are computed *transposed* (S^T[j,i] = K_j . Q_i) so that the
attention @ V matmul consumes the exp'd scores directly (contraction over j on
partitions) with no on-chip transposes.  Softmax tricks:
  - no max subtraction (scores are ~N(0,1); exp is safe in fp32)
  - padding mask + 1/sqrt(D) folded into Q at evacuation (masked query rows
    get Q=0 -> scores 0 -> exp 1 -> uniform attention over j<=i, exactly
    matching the reference's -1e9 fill + causal + softmax)
  - causal masking: off-diagonal key blocks skipped entirely; diagonal blocks
    get a 0/1 multiplicative mask post-exp
  - row sums via an all-ones [128,128] matmul accumulated alongside ctx
    (gives the sums replicated across partitions); fast-approx reciprocal
    then multiplies ctx at evacuation

Matmul dtype is float32r (TF32-like: full speed at free-dim>=256, ~1.4e-4
matmul relative error vs 2.1e-3 for bf16). In f32r mode phase A streams x
three times (V pass + two Q/K half passes) to fit SBUF.
"""

import sys

import numpy as np

sys.path.insert(0, "/opt/trn_rl_repo")

import ml_dtypes  # noqa: E402

import concourse.bacc as bacc_mod  # noqa: E402
import concourse.mybir as mybir  # noqa: E402
import concourse.tile as tile  # noqa: E402
from concourse.bass_utils import run_bass_kernel_spmd  # noqa: E402

B, T, C = 2, 2048, 2048
H = 16
D = 128
N_CORES = 8
HPC = 4          # heads per core
F = HPC * D      # 512: per-core feature slice of the C dim
P = 128
ITILE = 512      # i (query) tile width
IT = T // ITILE  # 4
CB = C // P      # 16 contraction blocks
TB = T // P      # 16 token blocks

MM_DTYPE = "float32r"   # "float32r" or "bfloat16"

_cache = {}


def _np_dt(name):
    return ml_dtypes.bfloat16 if name == "bfloat16" else np.float32


def _build_bass(mm_name):
    f32 = mybir.dt.float32
    mmdt = getattr(mybir.dt, mm_name)
    f32r_mode = mm_name == "float32r"
    mult = mybir.AluOpType.mult
    Exp = mybir.ActivationFunctionType.Exp

    nc = bacc_mod.Bacc("TRN2", target_bir_lowering=False, num_devices=N_CORES)

    xt_d = nc.dram_tensor("xt", [P, IT, CB, ITILE], mmdt, kind="ExternalInput")
    wq_d = nc.dram_tensor("wq", [P, CB, F], mmdt, kind="ExternalInput")
    wk_d = nc.dram_tensor("wk", [P, CB, F], mmdt, kind="ExternalInput")
    wv_d = nc.dram_tensor("wv", [P, CB, F], mmdt, kind="ExternalInput")
    wp_d = nc.dram_tensor("wp", [P, HPC, C], mmdt, kind="ExternalInput")
    mrep_d = nc.dram_tensor("mrep", [P, T], f32, kind="ExternalInput")
    cm_d = nc.dram_tensor("cm", [P, HPC, ITILE], mmdt, kind="ExternalInput")
    ones_d = nc.dram_tensor("ones_d", [P, P], mmdt, kind="ExternalInput")
    out_d = nc.dram_tensor("out", [P, TB, C], f32, kind="ExternalOutput")

    def proj_chain(psum, lhsT_fn, rhs_fn):
        for cb in range(CB):
            nc.tensor.matmul(psum[:], lhsT_fn(cb), rhs_fn(cb),
                             start=(cb == 0), stop=(cb == CB - 1))

    with tile.TileContext(nc) as tc:
        with tc.tile_pool(name="persist", bufs=1) as pers:
            if f32r_mode:
                # ---- Phase A-QK: Q,K projections, full Wq+Wk resident ----
                # (V comes after, so its SBUF isn't allocated yet and the
                # whole 2x4MB of Wq+Wk fits -> no weight-reload stalls)
                qk_cm = tc.tile_pool(name="qkpool", bufs=1)
                qkp = qk_cm.__enter__()
                qt = qkp.tile([P, HPC, T], mmdt)
                kt = qkp.tile([P, HPC, T], mmdt)
                with tc.tile_pool(name="wqk", bufs=1) as wqkp, \
                     tc.tile_pool(name="xpoolA", bufs=2) as xpA, \
                     tc.tile_pool(name="aqk_ps", bufs=2, space="PSUM") as paqk:
                    wq = wqkp.tile([P, CB, F], mmdt)
                    nc.sync.dma_start(wq[:], wq_d[:])
                    mrep = wqkp.tile([P, T], f32)
                    nc.sync.dma_start(mrep[:], mrep_d[:])
                    wk = wqkp.tile([P, CB, F], mmdt)
                    nc.sync.dma_start(wk[:], wk_d[:])
                    for it in range(IT):
                        isl = slice(it * ITILE, (it + 1) * ITILE)
                        xt = xpA.tile([P, CB, ITILE], mmdt, tag="xtA")
                        nc.sync.dma_start(xt[:], xt_d[:, it])
                        for db in range(HPC):
                            ps = paqk.tile([P, ITILE], f32, tag="pq")
                            proj_chain(ps,
                                       lambda cb: wq[:, cb, db * P:(db + 1) * P],
                                       lambda cb: xt[:, cb, :])
                            nc.vector.tensor_tensor(
                                qt[:, db, isl], ps[:], mrep[:, isl], mult)
                        for db in range(HPC):
                            ps = paqk.tile([P, ITILE], f32, tag="pq")
                            proj_chain(ps,
                                       lambda cb: wk[:, cb, db * P:(db + 1) * P],
                                       lambda cb: xt[:, cb, :])
                            nc.vector.tensor_copy(kt[:, db, isl], ps[:])

                # ---- Phase A-V: V projection ----
                vv_cm = tc.tile_pool(name="vvpool", bufs=1)
                vvp = vv_cm.__enter__()
                vv = vvp.tile([P, TB, F], mmdt)     # V  [j, (h,d)]
                with tc.tile_pool(name="wvpool", bufs=1) as wvp, \
                     tc.tile_pool(name="xpoolV", bufs=2) as xpV, \
                     tc.tile_pool(name="av_ps", bufs=2, space="PSUM") as pav:
                    wv = wvp.tile([P, CB, F], mmdt)
                    nc.sync.dma_start(wv[:], wv_d[:])
                    for it in range(IT):
                        xt = xpV.tile([P, CB, ITILE], mmdt, tag="xtV")
                        nc.sync.dma_start(xt[:], xt_d[:, it])
                        for tb in range(ITILE // P):
                            ps = pav.tile([P, F], f32, tag="pv")
                            proj_chain(ps,
                                       lambda cb: xt[:, cb, tb * P:(tb + 1) * P],
                                       lambda cb: wv[:, cb, :])
                            nc.vector.tensor_copy(
                                vv[:, it * (ITILE // P) + tb, :], ps[:])
            else:
                # ---- bf16: whole W resident, x streamed once ----
                vv_cm = None
                vv = pers.tile([P, TB, F], mmdt)     # V  [j, (h,d)]
                qk_cm = tc.tile_pool(name="qkpool", bufs=1)
                qkp = qk_cm.__enter__()
                qt = qkp.tile([P, HPC, T], mmdt)
                kt = qkp.tile([P, HPC, T], mmdt)
                with tc.tile_pool(name="wpool", bufs=1) as wpool, \
                     tc.tile_pool(name="xpool", bufs=2) as xpool, \
                     tc.tile_pool(name="pa_ps", bufs=2, space="PSUM") as pap:
                    mrep = wpool.tile([P, T], f32)
                    nc.sync.dma_start(mrep[:], mrep_d[:])
                    wq = wpool.tile([P, CB, F], mmdt)
                    wk = wpool.tile([P, CB, F], mmdt)
                    wv = wpool.tile([P, CB, F], mmdt)
                    nc.sync.dma_start(wq[:], wq_d[:])
                    nc.sync.dma_start(wk[:], wk_d[:])
                    nc.sync.dma_start(wv[:], wv_d[:])
                    for it in range(IT):
                        isl = slice(it * ITILE, (it + 1) * ITILE)
                        xt = xpool.tile([P, CB, ITILE], mmdt, tag="xt")
                        nc.sync.dma_start(xt[:], xt_d[:, it])
                        for db in range(HPC):
                            ps = pap.tile([P, ITILE], f32, tag="pp")
                            proj_chain(ps,
                                       lambda cb: wq[:, cb, db * P:(db + 1) * P],
                                       lambda cb: xt[:, cb, :])
                            nc.vector.tensor_tensor(
                                qt[:, db, isl], ps[:], mrep[:, isl], mult)
                        for db in range(HPC):
                            ps = pap.tile([P, ITILE], f32, tag="pp")
                            proj_chain(ps,
                                       lambda cb: wk[:, cb, db * P:(db + 1) * P],
                                       lambda cb: xt[:, cb, :])
                            nc.vector.tensor_copy(kt[:, db, isl], ps[:])
                        for tb in range(ITILE // P):
                            ps = pap.tile([P, ITILE], f32, tag="pp")
                            proj_chain(ps,
                                       lambda cb: xt[:, cb, tb * P:(tb + 1) * P],
                                       lambda cb: wv[:, cb, :])
                            nc.vector.tensor_copy(
                                vv[:, it * (ITILE // P) + tb, :], ps[:])

            # ---- Phase B: attention ----
            ctx_cm = tc.tile_pool(name="ctxpool", bufs=1)
            ctxp = ctx_cm.__enter__()
            ctx = ctxp.tile([P, HPC, T], mmdt)   # ctx^T per head [d, i]
            cmask = ctxp.tile([P, HPC, ITILE], mmdt)
            ones = ctxp.tile([P, P], mmdt)
            nc.sync.dma_start(cmask[:], cm_d[:])
            nc.sync.dma_start(ones[:], ones_d[:])
            wph = []
            for h in range(HPC):
                w = ctxp.tile([P, C], mmdt, name=f"wp{h}", tag=f"wp{h}")
                nc.sync.dma_start(w[:], wp_d[:, h])
                wph.append(w)
            pt_bufs = 1 if f32r_mode else 2
            with tc.tile_pool(name="ptpool", bufs=pt_bufs) as ptp, \
                 tc.tile_pool(name="bsb", bufs=2) as bsb, \
                 tc.tile_pool(name="st_ps", bufs=3, space="PSUM") as stp, \
                 tc.tile_pool(name="sum_ps", bufs=2, space="PSUM") as sump, \
                 tc.tile_pool(name="ctx_ps", bufs=2, space="PSUM") as ctxp:
                for h in range(HPC):
                    for it in range(IT):
                        njb = 4 * it + 4
                        isl = slice(it * ITILE, (it + 1) * ITILE)
                        pt = ptp.tile([P, TB, ITILE], mmdt, tag="pt")
                        ps_sum = sump.tile([P, ITILE], f32, tag="sum")
                        ps_ctx = ctxp.tile([P, ITILE], f32, tag="ctx")
                        for jb in range(njb):
                            ps_st = stp.tile([P, ITILE], f32, tag="st")
                            nc.tensor.matmul(
                                ps_st[:], kt[:, h, jb * P:(jb + 1) * P],
                                qt[:, h, isl], start=True, stop=True)
                            nc.scalar.activation(pt[:, jb], ps_st[:], Exp)
                            r = jb - 4 * it
                            if r >= 0:  # diagonal block: zero j > i entries
                                nc.vector.tensor_tensor(
                                    pt[:, jb], pt[:, jb], cmask[:, r], mult)
                            nc.tensor.matmul(
                                ps_ctx[:], vv[:, jb, h * P:(h + 1) * P], pt[:, jb],
                                start=(jb == 0), stop=(jb == njb - 1))
                        for jb in range(njb):
                            nc.tensor.matmul(
                                ps_sum[:], ones[:], pt[:, jb],
                                start=(jb == 0), stop=(jb == njb - 1))
                        rec = bsb.tile([P, ITILE], f32, tag="rec")
                        nc.vector.reciprocal_approx_fast(rec[:], ps_sum[:])
                        nc.vector.tensor_tensor(ctx[:, h, isl], ps_ctx[:], rec[:], mult)

            # ---- Phase C: output projection (partial over this core's heads) ----
            with tc.tile_pool(name="osb", bufs=2) as osb, \
                 tc.tile_pool(name="o_ps", bufs=2, space="PSUM") as ops:
                for tt in range(TB):
                    ot = osb.tile([P, C], f32, tag="ot")
                    for et in range(C // ITILE):
                        ps = ops.tile([P, ITILE], f32, tag="op")
                        for h in range(HPC):
                            nc.tensor.matmul(
                                ps[:], ctx[:, h, tt * P:(tt + 1) * P],
                                wph[h][:, et * ITILE:(et + 1) * ITILE],
                                start=(h == 0), stop=(h == HPC - 1))
                        nc.vector.tensor_copy(ot[:, et * ITILE:(et + 1) * ITILE], ps[:])
                    nc.sync.dma_start(out_d[:, tt], ot[:])

            ctx_cm.__exit__(None, None, None)
            if f32r_mode:
                vv_cm.__exit__(None, None, None)
            qk_cm.__exit__(None, None, None)

    nc.compile()
    return nc


def kernel(x, padding_mask, Wq, Wk, Wv, Wp, bp, **_unused):
    x = np.asarray(x, dtype=np.float32)
    padding_mask = np.asarray(padding_mask)
    Wq = np.asarray(Wq, dtype=np.float32)
    Wk = np.asarray(Wk, dtype=np.float32)
    Wv = np.asarray(Wv, dtype=np.float32)
    Wp = np.asarray(Wp, dtype=np.float32)
    bp = np.asarray(bp, dtype=np.float32)

    ndt = _np_dt(MM_DTYPE)

    if "nc" not in _cache:
        _cache["nc"] = _build_bass(MM_DTYPE)
    nc = _cache["nc"]

    # ---- host-side sharding / layout prep ----
    # xt[b]: [P, IT, CB, ITILE]; element [p,it,cb,u] = x[b, it*ITILE+u, cb*P+p]
    xt_b = []
    for b in range(B):
        xT = x[b].T  # [C, T]
        xt_b.append(np.ascontiguousarray(
            xT.reshape(CB, P, IT, ITILE).transpose(1, 2, 0, 3)).astype(ndt))

    def w_slice(W, f0):  # [P, CB, F] slice of W.T columns [f0, f0+F)
        return np.ascontiguousarray(
            W[f0:f0 + F, :].T.reshape(CB, P, F).transpose(1, 0, 2)).astype(ndt)

    # causal keep masks for the 4 diagonal sub-offsets
    pj = np.arange(P)[:, None]
    fi = np.arange(ITILE)[None, :]
    cm = np.stack([(fi >= pj + P * r) for r in range(HPC)], axis=1)  # [P,HPC,ITILE]
    cm = np.ascontiguousarray(cm).astype(ndt)
    ones_h = np.ones((P, P), dtype=np.float32).astype(ndt)

    mrep_b = []
    for b in range(B):
        mv = padding_mask[b].astype(np.float32) * (D ** -0.5)
        mrep_b.append(np.ascontiguousarray(
            np.broadcast_to(mv[None, :], (P, T))).astype(np.float32))

    in_maps = []
    for core in range(N_CORES):
        b = core // (N_CORES // B)
        hg = core % (N_CORES // B)
        f0 = hg * F
        in_maps.append({
            "xt": xt_b[b],
            "wq": w_slice(Wq, f0),
            "wk": w_slice(Wk, f0),
            "wv": w_slice(Wv, f0),
            "wp": np.ascontiguousarray(
                Wp[:, f0:f0 + F].T.reshape(HPC, P, C).transpose(1, 0, 2)).astype(ndt),
            "mrep": mrep_b[b],
            "cm": cm,
            "ones_d": ones_h,
        })

    kwargs = {}
    if _cache.get("trace"):
        kwargs = dict(trace=True, trace_cores=list(range(N_CORES)))
    res = run_bass_kernel_spmd(nc, in_maps, core_ids=list(range(N_CORES)), **kwargs)
    _cache["last_results"] = res

    out = np.zeros((B, T, C), dtype=np.float32)
    for core in range(N_CORES):
        b = core // (N_CORES // B)
        o = res.results[core]["out"]  # [P, TB, C]
        out[b] += o.transpose(1, 0, 2).reshape(T, C)
    out += bp[None, None, :]
    return out


# revision 13
# speedup vs baseline: 1.0241x; 1.0241x over previous
"""Multi-head causal attention (B=2, T=2048, C=2048, H=16) on 8 TRN2 NeuronCores.

Sharding: data parallel over batch (2) x tensor parallel over head groups
(16 heads -> 4 groups of 4). Core c handles batch c//4, heads 4*(c%4)..4*(c%4)+3.
Each core computes its Megatron-style partial output projection; the host sums
the 4 partials per batch and adds the bias.

All matmuls are natural-layout because the host pre-transposes x and the
weights.  Scores are computed *transposed* (S^T[j,i] = K_j . Q_i) so that the
attention @ V matmul consumes the exp'd scores directly (contraction over j on
partitions) with no on-chip transposes.  Softmax tricks:
  - no max subtraction (scores are ~N(0,1); exp is safe in fp32)
  - padding mask + 1/sqrt(D) folded into Q at evacuation (masked query rows
    get Q=0 -> scores 0 -> exp 1 -> uniform attention over j<=i, exactly
    matching the reference's -1e9 fill + causal + softmax)
  - causal masking: off-diagonal key blocks skipped entirely; diagonal blocks
    get a 0/1 multiplicative mask post-exp
  - row sums via an all-ones [128,128] matmul accumulated alongside ctx
    (gives the sums replicated across partitions); fast-approx reciprocal
    then multiplies ctx at evacuation

Matmul dtype is float32r (TF32-like: full speed at free-dim>=256, ~1.4e-4
matmul relative error vs 2.1e-3 for bf16). In f32r mode phase A streams x
three times (V pass + two Q/K half passes) to fit SBUF.
"""

import sys

import numpy as np

sys.path.insert(0, "/opt/trn_rl_repo")

import ml_dtypes  # noqa: E402

import concourse.bacc as bacc_mod  # noqa: E402
import concourse.bass as bass  # noqa: E402
import concourse.mybir as mybir  # noqa: E402
import concourse.tile as tile  # noqa: E402
from concourse.bass_utils import run_bass_kernel_spmd  # noqa: E402

B, T, C = 2, 2048, 2048
H = 16
D = 128
N_CORES = 8
HPC = 4          # heads per core
F = HPC * D      # 512: per-core feature slice of the C dim
P = 128
ITILE = 512      # i (query) tile width
IT = T // ITILE  # 4
CB = C // P      # 16 contraction blocks
TB = T // P      # 16 token blocks

MM_DTYPE = "float32r"   # "float32r" or "bfloat16"

_cache = {}


def _np_dt(name):
    return ml_dtypes.bfloat16 if name == "bfloat16" else np.float32


def _build_bass(mm_name):
    f32 = mybir.dt.float32
    mmdt = getattr(mybir.dt, mm_name)
    f32r_mode = mm_name == "float32r"
    mult = mybir.AluOpType.mult
    Exp = mybir.ActivationFunctionType.Exp

    nc = bacc_mod.Bacc("TRN2", target_bir_lowering=False, num_devices=N_CORES)

    xt_d = nc.dram_tensor("xt", [P, IT, CB, ITILE], mmdt, kind="ExternalInput")
    wq_d = nc.dram_tensor("wq", [P, CB, F], mmdt, kind="ExternalInput")
    wk_d = nc.dram_tensor("wk", [P, CB, F], mmdt, kind="ExternalInput")
    wv_d = nc.dram_tensor("wv", [P, CB, F], mmdt, kind="ExternalInput")
    wp_d = nc.dram_tensor("wp", [P, HPC, C], mmdt, kind="ExternalInput")
    mrep_d = nc.dram_tensor("mrep", [P, T], f32, kind="ExternalInput")
    cm_d = nc.dram_tensor("cm", [P, HPC, ITILE], mmdt, kind="ExternalInput")
    ones_d = nc.dram_tensor("ones_d", [P, P], mmdt, kind="ExternalInput")
    out_d = nc.dram_tensor("out", [P, TB, C], f32, kind="ExternalOutput")

    def proj_chain(psum, lhsT_fn, rhs_fn):
        for cb in range(CB):
            nc.tensor.matmul(psum[:], lhsT_fn(cb), rhs_fn(cb),
                             start=(cb == 0), stop=(cb == CB - 1))

    with tile.TileContext(nc) as tc:
        with tc.tile_pool(name="persist", bufs=1) as pers:
            if f32r_mode:
                # Phase A, fp32r: V pass first (low SBUF pressure) with the
                # Q/K half-weights + mrep prefetching underneath it; one x
                # pool shared by all three passes so x-tile DMAs roll
                # continuously across pass boundaries.
                F2 = F // 2
                qk_cm = tc.tile_pool(name="qkpool", bufs=1, side="right")   # entered later
                vv_cm = tc.tile_pool(name="vvpool", bufs=1, side="right")
                vvp = vv_cm.__enter__()
                vv = vvp.tile([P, TB, F], mmdt)     # V  [j, (h,d)]
                with tc.tile_pool(name="wqk", bufs=1) as wqkp, \
                     tc.tile_pool(name="xpoolA", bufs=2) as xpA, \
                     tc.tile_pool(name="a_ps", bufs=2, space="PSUM") as pa:
                    # prefetch first-half Q/K weights + mask under the V pass
                    wqh = wqkp.tile([P, CB, F2], mmdt, tag="wqh")
                    nc.sync.dma_start(wqh[:], wq_d[:, :, 0:F2])
                    wkh = wqkp.tile([P, CB, F2], mmdt, tag="wkh")
                    nc.sync.dma_start(wkh[:], wk_d[:, :, 0:F2])
                    mrep = wqkp.tile([P, T], f32)
                    nc.sync.dma_start(mrep[:], mrep_d[:])

                    with tc.tile_pool(name="wvpool", bufs=1) as wvp:
                        wv = wvp.tile([P, CB, F], mmdt)
                        nc.sync.dma_start(wv[:], wv_d[:])
                        for it in range(IT):
                            xt = xpA.tile([P, CB, ITILE], mmdt, tag="xtA")
                            nc.sync.dma_start(xt[:], xt_d[:, it])
                            for tb in range(ITILE // P):
                                ps = pa.tile([P, F], f32, tag="pp")
                                proj_chain(ps,
                                           lambda cb: xt[:, cb, tb * P:(tb + 1) * P],
                                           lambda cb: wv[:, cb, :])
                                nc.vector.tensor_copy(
                                    vv[:, it * (ITILE // P) + tb, :], ps[:])

                    qkp = qk_cm.__enter__()
                    qt = qkp.tile([P, HPC, T], mmdt)
                    kt = qkp.tile([P, HPC, T], mmdt)
                    for wh in range(2):
                        if wh > 0:
                            wqh = wqkp.tile([P, CB, F2], mmdt, tag="wqh")
                            nc.sync.dma_start(
                                wqh[:], wq_d[:, :, wh * F2:(wh + 1) * F2])
                            wkh = wqkp.tile([P, CB, F2], mmdt, tag="wkh")
                            nc.sync.dma_start(
                                wkh[:], wk_d[:, :, wh * F2:(wh + 1) * F2])
                        for it in range(IT):
                            isl = slice(it * ITILE, (it + 1) * ITILE)
                            xt = xpA.tile([P, CB, ITILE], mmdt, tag="xtA")
                            nc.sync.dma_start(xt[:], xt_d[:, it])
                            for db in range(2):
                                h = wh * 2 + db
                                ps = pa.tile([P, ITILE], f32, tag="pp")
                                proj_chain(ps,
                                           lambda cb: wqh[:, cb, db * P:(db + 1) * P],
                                           lambda cb: xt[:, cb, :])
                                nc.vector.tensor_tensor(
                                    qt[:, h, isl], ps[:], mrep[:, isl], mult)
                            for db in range(2):
                                h = wh * 2 + db
                                ps = pa.tile([P, ITILE], f32, tag="pp")
                                proj_chain(ps,
                                           lambda cb: wkh[:, cb, db * P:(db + 1) * P],
                                           lambda cb: xt[:, cb, :])
                                nc.vector.tensor_copy(kt[:, h, isl], ps[:])
            else:
                # ---- bf16: whole W resident, x streamed once ----
                vv_cm = None
                vv = pers.tile([P, TB, F], mmdt)     # V  [j, (h,d)]
                qk_cm = tc.tile_pool(name="qkpool", bufs=1, side="right")
                qkp = qk_cm.__enter__()
                qt = qkp.tile([P, HPC, T], mmdt)
                kt = qkp.tile([P, HPC, T], mmdt)
                with tc.tile_pool(name="wpool", bufs=1) as wpool, \
                     tc.tile_pool(name="xpool", bufs=2) as xpool, \
                     tc.tile_pool(name="pa_ps", bufs=2, space="PSUM") as pap:
                    mrep = wpool.tile([P, T], f32)
                    nc.sync.dma_start(mrep[:], mrep_d[:])
                    wq = wpool.tile([P, CB, F], mmdt)
                    wk = wpool.tile([P, CB, F], mmdt)
                    wv = wpool.tile([P, CB, F], mmdt)
                    nc.sync.dma_start(wq[:], wq_d[:])
                    nc.sync.dma_start(wk[:], wk_d[:])
                    nc.sync.dma_start(wv[:], wv_d[:])
                    for it in range(IT):
                        isl = slice(it * ITILE, (it + 1) * ITILE)
                        xt = xpool.tile([P, CB, ITILE], mmdt, tag="xt")
                        nc.sync.dma_start(xt[:], xt_d[:, it])
                        for db in range(HPC):
                            ps = pap.tile([P, ITILE], f32, tag="pp")
                            proj_chain(ps,
                                       lambda cb: wq[:, cb, db * P:(db + 1) * P],
                                       lambda cb: xt[:, cb, :])
                            nc.vector.tensor_tensor(
                                qt[:, db, isl], ps[:], mrep[:, isl], mult)
                        for db in range(HPC):
                            ps = pap.tile([P, ITILE], f32, tag="pp")
                            proj_chain(ps,
                                       lambda cb: wk[:, cb, db * P:(db + 1) * P],
                                       lambda cb: xt[:, cb, :])
                            nc.vector.tensor_copy(kt[:, db, isl], ps[:])
                        for tb in range(ITILE // P):
                            ps = pap.tile([P, ITILE], f32, tag="pp")
                            proj_chain(ps,
                                       lambda cb: xt[:, cb, tb * P:(tb + 1) * P],
                                       lambda cb: wv[:, cb, :])
                            nc.vector.tensor_copy(
                                vv[:, it * (ITILE // P) + tb, :], ps[:])

            # ---- Phase B: attention ----
            ctx_cm = tc.tile_pool(name="ctxpool", bufs=1, side="right")
            ctxp = ctx_cm.__enter__()
            ctx = ctxp.tile([P, HPC, T], mmdt)   # ctx^T per head [d, i]
            cmask = ctxp.tile([P, HPC, ITILE], mmdt)
            ones = ctxp.tile([P, P], mmdt)
            nc.sync.dma_start(cmask[:], cm_d[:])
            nc.sync.dma_start(ones[:], ones_d[:])
            wph = []
            for h in range(HPC):
                w = ctxp.tile([P, C], mmdt, name=f"wp{h}", tag=f"wp{h}")
                nc.sync.dma_start(w[:], wp_d[:, h])
                wph.append(w)
            pt_bufs = 1 if f32r_mode else 2
            with tc.tile_pool(name="ptpool", bufs=pt_bufs) as ptp, \
                 tc.tile_pool(name="bsb", bufs=2) as bsb, \
                 tc.tile_pool(name="st_ps", bufs=4, space="PSUM") as stp, \
                 tc.tile_pool(name="sum_ps", bufs=2, space="PSUM") as sump, \
                 tc.tile_pool(name="ctx_ps", bufs=2, space="PSUM") as ctxp:
                for h in range(HPC):
                    for it in range(IT):
                        njb = 4 * it + 4
                        isl = slice(it * ITILE, (it + 1) * ITILE)
                        pt = ptp.tile([P, TB, ITILE], mmdt, tag="pt")
                        ps_sum = sump.tile([P, ITILE], f32, tag="sum")
                        ps_ctx = ctxp.tile([P, ITILE], f32, tag="ctx")
                        for jb in range(njb):
                            ps_st = stp.tile([P, ITILE], f32, tag="st")
                            nc.tensor.matmul(
                                ps_st[:], kt[:, h, jb * P:(jb + 1) * P],
                                qt[:, h, isl], start=True, stop=True)
                            nc.scalar.activation(pt[:, jb], ps_st[:], Exp)
                            r = jb - 4 * it
                            if r >= 0:  # diagonal block: zero j > i entries
                                nc.vector.tensor_tensor(
                                    pt[:, jb], pt[:, jb], cmask[:, r], mult)
                            nc.tensor.matmul(
                                ps_ctx[:], vv[:, jb, h * P:(h + 1) * P], pt[:, jb],
                                start=(jb == 0), stop=(jb == njb - 1))
                        for jb in range(njb):
                            nc.tensor.matmul(
                                ps_sum[:], ones[:], pt[:, jb],
                                start=(jb == 0), stop=(jb == njb - 1))
                        rec = bsb.tile([P, ITILE], f32, tag="rec")
                        nc.vector.reciprocal_approx_fast(rec[:], ps_sum[:])
                        nc.vector.tensor_tensor(ctx[:, h, isl], ps_ctx[:], rec[:], mult)

            # ---- Phase C: output projection (partial over this core's heads) ----
            with tc.tile_pool(name="osb", bufs=2) as osb, \
                 tc.tile_pool(name="o_ps", bufs=2, space="PSUM") as ops:
                for tt in range(TB):
                    ot = osb.tile([P, C], f32, tag="ot")
                    for et in range(C // ITILE):
                        ps = ops.tile([P, ITILE], f32, tag="op")
                        for h in range(HPC):
                            nc.tensor.matmul(
                                ps[:], ctx[:, h, tt * P:(tt + 1) * P],
                                wph[h][:, et * ITILE:(et + 1) * ITILE],
                                start=(h == 0), stop=(h == HPC - 1))
                        nc.vector.tensor_copy(ot[:, et * ITILE:(et + 1) * ITILE], ps[:])
                    nc.sync.dma_start(out_d[:, tt], ot[:])

            ctx_cm.__exit__(None, None, None)
            qk_cm.__exit__(None, None, None)
            if f32r_mode:
                vv_cm.__exit__(None, None, None)

    nc.compile()
    return nc


def kernel(x, padding_mask, Wq, Wk, Wv, Wp, bp, **_unused):
    x = np.asarray(x, dtype=np.float32)
    padding_mask = np.asarray(padding_mask)
    Wq = np.asarray(Wq, dtype=np.float32)
    Wk = np.asarray(Wk, dtype=np.float32)
    Wv = np.asarray(Wv, dtype=np.float32)
    Wp = np.asarray(Wp, dtype=np.float32)
    bp = np.asarray(bp, dtype=np.float32)

    ndt = _np_dt(MM_DTYPE)

    if "nc" not in _cache:
        _cache["nc"] = _build_bass(MM_DTYPE)
    nc = _cache["nc"]

    # ---- host-side sharding / layout prep ----
    # xt[b]: [P, IT, CB, ITILE]; element [p,it,cb,u] = x[b, it*ITILE+u, cb*P+p]
    xt_b = []
    for b in range(B):
        xT = x[b].T  # [C, T]
        xt_b.append(np.ascontiguousarray(
            xT.reshape(CB, P, IT, ITILE).transpose(1, 2, 0, 3)).astype(ndt))

    def w_slice(W, f0):  # [P, CB, F] slice of W.T columns [f0, f0+F)
        return np.ascontiguousarray(
            W[f0:f0 + F, :].T.reshape(CB, P, F).transpose(1, 0, 2)).astype(ndt)

    # causal keep masks for the 4 diagonal sub-offsets
    pj = np.arange(P)[:, None]
    fi = np.arange(ITILE)[None, :]
    cm = np.stack([(fi >= pj + P * r) for r in range(HPC)], axis=1)  # [P,HPC,ITILE]
    cm = np.ascontiguousarray(cm).astype(ndt)
    ones_h = np.ones((P, P), dtype=np.float32).astype(ndt)

    mrep_b = []
    for b in range(B):
        mv = padding_mask[b].astype(np.float32) * (D ** -0.5)
        mrep_b.append(np.ascontiguousarray(
            np.broadcast_to(mv[None, :], (P, T))).astype(np.float32))

    in_maps = []
    for core in range(N_CORES):
        b = core // (N_CORES // B)
        hg = core % (N_CORES // B)
        f0 = hg * F
        in_maps.append({
            "xt": xt_b[b],
            "wq": w_slice(Wq, f0),
            "wk": w_slice(Wk, f0),
            "wv": w_slice(Wv, f0),
            "wp": np.ascontiguousarray(
                Wp[:, f0:f0 + F].T.reshape(HPC, P, C).transpose(1, 0, 2)).astype(ndt),
            "mrep": mrep_b[b],
            "cm": cm,
            "ones_d": ones_h,
        })

    kwargs = {}
    if _cache.get("trace"):
        kwargs = dict(trace=True, trace_cores=list(range(N_CORES)))
    res = run_bass_kernel_spmd(nc, in_maps, core_ids=list(range(N_CORES)), **kwargs)
    _cache["last_results"] = res

    out = np.zeros((B, T, C), dtype=np.float32)
    for core in range(N_CORES):
        b = core // (N_CORES // B)
        o = res.results[core]["out"]  # [P, TB, C]
        out[b] += o.transpose(1, 0, 2).reshape(T, C)
    out += bp[None, None, :]
    return out


# revision 14
# speedup vs baseline: 1.0513x; 1.0266x over previous
"""Multi-head causal attention (B=2, T=2048, C=2048, H=16) on 8 TRN2 NeuronCores.

Sharding: data parallel over batch (2) x tensor parallel over head groups
(16 heads -> 4 groups of 4). Core c handles batch c//4, heads 4*(c%4)..4*(c%4)+3.
Each core computes its Megatron-style partial output projection; the host sums
the 4 partials per batch and adds the bias.

All matmuls are natural-layout because the host pre-transposes x and the
weights.  Scores are computed *transposed* (S^T[j,i] = K_j . Q_i) so that the
attention @ V matmul consumes the exp'd scores directly (contraction over j on
partitions) with no on-chip transposes.  Softmax tricks:
  - no max subtraction (scores are ~N(0,1); exp is safe in fp32)
  - padding mask + 1/sqrt(D) folded into Q at evacuation (masked query rows
    get Q=0 -> scores 0 -> exp 1 -> uniform attention over j<=i, exactly
    matching the reference's -1e9 fill + causal + softmax)
  - causal masking: off-diagonal key blocks skipped entirely; diagonal blocks
    get a 0/1 multiplicative mask post-exp
  - row sums via an all-ones [128,128] matmul accumulated alongside ctx
    (gives the sums replicated across partitions); fast-approx reciprocal
    then multiplies ctx at evacuation

Matmul dtype is float32r (TF32-like: full speed at free-dim>=256, ~1.4e-4
matmul relative error vs 2.1e-3 for bf16). In f32r mode phase A streams x
three times (V pass + two Q/K half passes) to fit SBUF.
"""

import sys

import numpy as np

sys.path.insert(0, "/opt/trn_rl_repo")

import ml_dtypes  # noqa: E402

import concourse.bacc as bacc_mod  # noqa: E402
import concourse.bass as bass  # noqa: E402
import concourse.mybir as mybir  # noqa: E402
import concourse.tile as tile  # noqa: E402
from concourse.bass_utils import run_bass_kernel_spmd  # noqa: E402

B, T, C = 2, 2048, 2048
H = 16
D = 128
N_CORES = 8
HPC = 4          # heads per core
F = HPC * D      # 512: per-core feature slice of the C dim
P = 128
ITILE = 512      # i (query) tile width
IT = T // ITILE  # 4
CB = C // P      # 16 contraction blocks
TB = T // P      # 16 token blocks

MM_DTYPE = "float32r"   # "float32r" or "bfloat16"

_cache = {}


def _np_dt(name):
    return ml_dtypes.bfloat16 if name == "bfloat16" else np.float32


def _build_bass(mm_name):
    f32 = mybir.dt.float32
    mmdt = getattr(mybir.dt, mm_name)
    f32r_mode = mm_name == "float32r"
    mult = mybir.AluOpType.mult
    Exp = mybir.ActivationFunctionType.Exp

    nc = bacc_mod.Bacc("TRN2", target_bir_lowering=False, num_devices=N_CORES)

    xt_d = nc.dram_tensor("xt", [P, IT, CB, ITILE], mmdt, kind="ExternalInput")
    wq_d = nc.dram_tensor("wq", [P, CB, F], mmdt, kind="ExternalInput")
    wk_d = nc.dram_tensor("wk", [P, CB, F], mmdt, kind="ExternalInput")
    wv_d = nc.dram_tensor("wv", [P, CB, F], mmdt, kind="ExternalInput")
    wp_d = nc.dram_tensor("wp", [P, HPC, C], mmdt, kind="ExternalInput")
    mrep_d = nc.dram_tensor("mrep", [P, T], f32, kind="ExternalInput")
    cm_d = nc.dram_tensor("cm", [P, HPC, ITILE], mmdt, kind="ExternalInput")
    ones_d = nc.dram_tensor("ones_d", [P, P], mmdt, kind="ExternalInput")
    out_d = nc.dram_tensor("out", [P, TB, C], f32, kind="ExternalOutput")

    def proj_chain(psum, lhsT_fn, rhs_fn):
        for cb in range(CB):
            nc.tensor.matmul(psum[:], lhsT_fn(cb), rhs_fn(cb),
                             start=(cb == 0), stop=(cb == CB - 1))

    with tile.TileContext(nc) as tc:
        with tc.tile_pool(name="persist", bufs=1) as pers:
            if f32r_mode:
                # Phase A, fp32r: V pass first (low SBUF pressure) with the
                # Q/K half-weights + mrep prefetching underneath it; one x
                # pool shared by all three passes so x-tile DMAs roll
                # continuously across pass boundaries.
                F2 = F // 2
                qk_cm = tc.tile_pool(name="qkpool", bufs=1, side="right")   # entered later
                vv_cm = tc.tile_pool(name="vvpool", bufs=1, side="right")
                vvp = vv_cm.__enter__()
                vv = vvp.tile([P, TB, F], mmdt)     # V  [j, (h,d)]
                with tc.tile_pool(name="wqk", bufs=1) as wqkp, \
                     tc.tile_pool(name="xpoolA", bufs=2) as xpA, \
                     tc.tile_pool(name="a_ps", bufs=2, space="PSUM") as pa:
                    # prefetch first-half Q/K weights + mask under the V pass
                    wqh = wqkp.tile([P, CB, F2], mmdt, tag="wqh")
                    nc.sync.dma_start(wqh[:], wq_d[:, :, 0:F2])
                    wkh = wqkp.tile([P, CB, F2], mmdt, tag="wkh")
                    nc.sync.dma_start(wkh[:], wk_d[:, :, 0:F2])
                    mrep = wqkp.tile([P, T], f32)
                    nc.sync.dma_start(mrep[:], mrep_d[:])

                    with tc.tile_pool(name="wvpool", bufs=1) as wvp:
                        wv = wvp.tile([P, CB, F], mmdt)
                        nc.sync.dma_start(wv[:], wv_d[:])
                        for it in range(IT):
                            xt = xpA.tile([P, CB, ITILE], mmdt, tag="xtA")
                            nc.sync.dma_start(xt[:], xt_d[:, it])
                            for tb in range(ITILE // P):
                                ps = pa.tile([P, F], f32, tag="pp")
                                proj_chain(ps,
                                           lambda cb: xt[:, cb, tb * P:(tb + 1) * P],
                                           lambda cb: wv[:, cb, :])
                                nc.vector.tensor_copy(
                                    vv[:, it * (ITILE // P) + tb, :], ps[:])

                    qkp = qk_cm.__enter__()
                    qt = qkp.tile([P, HPC, T], mmdt)
                    kt = qkp.tile([P, HPC, T], mmdt)
                    for wh in range(2):
                        if wh > 0:
                            wqh = wqkp.tile([P, CB, F2], mmdt, tag="wqh")
                            nc.sync.dma_start(
                                wqh[:], wq_d[:, :, wh * F2:(wh + 1) * F2])
                            wkh = wqkp.tile([P, CB, F2], mmdt, tag="wkh")
                            nc.sync.dma_start(
                                wkh[:], wk_d[:, :, wh * F2:(wh + 1) * F2])
                        for it in range(IT):
                            isl = slice(it * ITILE, (it + 1) * ITILE)
                            xt = xpA.tile([P, CB, ITILE], mmdt, tag="xtA")
                            nc.sync.dma_start(xt[:], xt_d[:, it])
                            for db in range(2):
                                h = wh * 2 + db
                                ps = pa.tile([P, ITILE], f32, tag="pp")
                                proj_chain(ps,
                                           lambda cb: wqh[:, cb, db * P:(db + 1) * P],
                                           lambda cb: xt[:, cb, :])
                                nc.vector.tensor_tensor(
                                    qt[:, h, isl], ps[:], mrep[:, isl], mult)
                            for db in range(2):
                                h = wh * 2 + db
                                ps = pa.tile([P, ITILE], f32, tag="pp")
                                proj_chain(ps,
                                           lambda cb: wkh[:, cb, db * P:(db + 1) * P],
                                           lambda cb: xt[:, cb, :])
                                nc.vector.tensor_copy(kt[:, h, isl], ps[:])
            else:
                # ---- bf16: whole W resident, x streamed once ----
                vv_cm = None
                vv = pers.tile([P, TB, F], mmdt)     # V  [j, (h,d)]
                qk_cm = tc.tile_pool(name="qkpool", bufs=1, side="right")
                qkp = qk_cm.__enter__()
                qt = qkp.tile([P, HPC, T], mmdt)
                kt = qkp.tile([P, HPC, T], mmdt)
                with tc.tile_pool(name="wpool", bufs=1) as wpool, \
                     tc.tile_pool(name="xpool", bufs=2) as xpool, \
                     tc.tile_pool(name="pa_ps", bufs=2, space="PSUM") as pap:
                    mrep = wpool.tile([P, T], f32)
                    nc.sync.dma_start(mrep[:], mrep_d[:])
                    wq = wpool.tile([P, CB, F], mmdt)
                    wk = wpool.tile([P, CB, F], mmdt)
                    wv = wpool.tile([P, CB, F], mmdt)
                    nc.sync.dma_start(wq[:], wq_d[:])
                    nc.sync.dma_start(wk[:], wk_d[:])
                    nc.sync.dma_start(wv[:], wv_d[:])
                    for it in range(IT):
                        isl = slice(it * ITILE, (it + 1) * ITILE)
                        xt = xpool.tile([P, CB, ITILE], mmdt, tag="xt")
                        nc.sync.dma_start(xt[:], xt_d[:, it])
                        for db in range(HPC):
                            ps = pap.tile([P, ITILE], f32, tag="pp")
                            proj_chain(ps,
                                       lambda cb: wq[:, cb, db * P:(db + 1) * P],
                                       lambda cb: xt[:, cb, :])
                            nc.vector.tensor_tensor(
                                qt[:, db, isl], ps[:], mrep[:, isl], mult)
                        for db in range(HPC):
                            ps = pap.tile([P, ITILE], f32, tag="pp")
                            proj_chain(ps,
                                       lambda cb: wk[:, cb, db * P:(db + 1) * P],
                                       lambda cb: xt[:, cb, :])
                            nc.vector.tensor_copy(kt[:, db, isl], ps[:])
                        for tb in range(ITILE // P):
                            ps = pap.tile([P, ITILE], f32, tag="pp")
                            proj_chain(ps,
                                       lambda cb: xt[:, cb, tb * P:(tb + 1) * P],
                                       lambda cb: wv[:, cb, :])
                            nc.vector.tensor_copy(
                                vv[:, it * (ITILE // P) + tb, :], ps[:])

            # ---- Phase B: attention ----
            ctx_cm = tc.tile_pool(name="ctxpool", bufs=1, side="right")
            ctxp = ctx_cm.__enter__()
            ctx = ctxp.tile([P, HPC, T], mmdt)   # ctx^T per head [d, i]
            cmask = ctxp.tile([P, HPC, ITILE], mmdt)
            ones = ctxp.tile([P, P], mmdt)
            nc.sync.dma_start(cmask[:], cm_d[:])
            nc.sync.dma_start(ones[:], ones_d[:])
            wph = []
            for h in range(HPC):
                w = ctxp.tile([P, C], mmdt, name=f"wp{h}", tag=f"wp{h}")
                nc.sync.dma_start(w[:], wp_d[:, h])
                wph.append(w)
            pt_bufs = 1 if f32r_mode else 2
            with tc.tile_pool(name="ptpool", bufs=pt_bufs) as ptp, \
                 tc.tile_pool(name="bsb", bufs=2) as bsb, \
                 tc.tile_pool(name="st_ps", bufs=4, space="PSUM") as stp, \
                 tc.tile_pool(name="sum_ps", bufs=2, space="PSUM") as sump, \
                 tc.tile_pool(name="ctx_ps", bufs=2, space="PSUM") as ctxp:
                for h in range(HPC):
                    for it in range(IT):
                        njb = 4 * it + 4
                        isl = slice(it * ITILE, (it + 1) * ITILE)
                        pt = ptp.tile([P, TB, ITILE], mmdt, tag="pt")
                        ps_sum = sump.tile([P, ITILE], f32, tag="sum")
                        ps_ctx = ctxp.tile([P, ITILE], f32, tag="ctx")
                        for jb in range(njb):
                            ps_st = stp.tile([P, ITILE], f32, tag="st")
                            nc.tensor.matmul(
                                ps_st[:], kt[:, h, jb * P:(jb + 1) * P],
                                qt[:, h, isl], start=True, stop=True)
                            nc.scalar.activation(pt[:, jb], ps_st[:], Exp)
                            r = jb - 4 * it
                            if r >= 0:  # diagonal block: zero j > i entries
                                nc.vector.tensor_tensor(
                                    pt[:, jb], pt[:, jb], cmask[:, r], mult)
                            nc.tensor.matmul(
                                ps_ctx[:], vv[:, jb, h * P:(h + 1) * P], pt[:, jb],
                                start=(jb == 0), stop=(jb == njb - 1))
                        for jb in range(njb):
                            nc.tensor.matmul(
                                ps_sum[:], ones[:], pt[:, jb],
                                start=(jb == 0), stop=(jb == njb - 1))
                        rec = bsb.tile([P, ITILE], f32, tag="rec")
                        nc.vector.reciprocal_approx_fast(rec[:], ps_sum[:])
                        nc.vector.tensor_tensor(ctx[:, h, isl], ps_ctx[:], rec[:], mult)

            # ---- Phase C: output projection (partial over this core's heads) ----
            with tc.tile_pool(name="osb", bufs=3) as osb, \
                 tc.tile_pool(name="o_ps", bufs=4, space="PSUM") as ops:
                for tt in range(TB):
                    ot = osb.tile([P, C], f32, tag="ot")
                    for et in range(C // ITILE):
                        ps = ops.tile([P, ITILE], f32, tag="op")
                        for h in range(HPC):
                            nc.tensor.matmul(
                                ps[:], ctx[:, h, tt * P:(tt + 1) * P],
                                wph[h][:, et * ITILE:(et + 1) * ITILE],
                                start=(h == 0), stop=(h == HPC - 1))
                        nc.vector.tensor_copy(ot[:, et * ITILE:(et + 1) * ITILE], ps[:])
                    nc.sync.dma_start(out_d[:, tt], ot[:])

            ctx_cm.__exit__(None, None, None)
            qk_cm.__exit__(None, None, None)
            if f32r_mode:
                vv_cm.__exit__(None, None, None)

    nc.compile()
    return nc


def kernel(x, padding_mask, Wq, Wk, Wv, Wp, bp, **_unused):
    x = np.asarray(x, dtype=np.float32)
    padding_mask = np.asarray(padding_mask)
    Wq = np.asarray(Wq, dtype=np.float32)
    Wk = np.asarray(Wk, dtype=np.float32)
    Wv = np.asarray(Wv, dtype=np.float32)
    Wp = np.asarray(Wp, dtype=np.float32)
    bp = np.asarray(bp, dtype=np.float32)

    ndt = _np_dt(MM_DTYPE)

    if "nc" not in _cache:
        _cache["nc"] = _build_bass(MM_DTYPE)
    nc = _cache["nc"]

    # ---- host-side sharding / layout prep ----
    # xt[b]: [P, IT, CB, ITILE]; element [p,it,cb,u] = x[b, it*ITILE+u, cb*P+p]
    xt_b = []
    for b in range(B):
        xT = x[b].T  # [C, T]
        xt_b.append(np.ascontiguousarray(
            xT.reshape(CB, P, IT, ITILE).transpose(1, 2, 0, 3)).astype(ndt))

    def w_slice(W, f0):  # [P, CB, F] slice of W.T columns [f0, f0+F)
        return np.ascontiguousarray(
            W[f0:f0 + F, :].T.reshape(CB, P, F).transpose(1, 0, 2)).astype(ndt)

    # causal keep masks for the 4 diagonal sub-offsets
    pj = np.arange(P)[:, None]
    fi = np.arange(ITILE)[None, :]
    cm = np.stack([(fi >= pj + P * r) for r in range(HPC)], axis=1)  # [P,HPC,ITILE]
    cm = np.ascontiguousarray(cm).astype(ndt)
    ones_h = np.ones((P, P), dtype=np.float32).astype(ndt)

    mrep_b = []
    for b in range(B):
        mv = padding_mask[b].astype(np.float32) * (D ** -0.5)
        mrep_b.append(np.ascontiguousarray(
            np.broadcast_to(mv[None, :], (P, T))).astype(np.float32))

    in_maps = []
    for core in range(N_CORES):
        b = core // (N_CORES // B)
        hg = core % (N_CORES // B)
        f0 = hg * F
        in_maps.append({
            "xt": xt_b[b],
            "wq": w_slice(Wq, f0),
            "wk": w_slice(Wk, f0),
            "wv": w_slice(Wv, f0),
            "wp": np.ascontiguousarray(
                Wp[:, f0:f0 + F].T.reshape(HPC, P, C).transpose(1, 0, 2)).astype(ndt),
            "mrep": mrep_b[b],
            "cm": cm,
            "ones_d": ones_h,
        })

    kwargs = {}
    if _cache.get("trace"):
        kwargs = dict(trace=True, trace_cores=list(range(N_CORES)))
    res = run_bass_kernel_spmd(nc, in_maps, core_ids=list(range(N_CORES)), **kwargs)
    _cache["last_results"] = res

    out = np.zeros((B, T, C), dtype=np.float32)
    for core in range(N_CORES):
        b = core // (N_CORES // B)
        o = res.results[core]["out"]  # [P, TB, C]
        out[b] += o.transpose(1, 0, 2).reshape(T, C)
    out += bp[None, None, :]
    return out
